# revision 11
# baseline (speedup 1.0000x reference)
"""Causal self-attention Trainium2 Bass kernel.

Shapes (hardcoded): B=8, T=1024, C=768, NH=12, HS=64.
Sharding: data-parallel over batch — core b computes batch element b.

Per-core dataflow (all matmuls in float32r — tf32-like, ~1.5e-4 rel err):
  - qkT  [2C, T] channel-major  = w_qk.T-tiles (stationary) x xT (moving)
  - v    token-major [T, C], assembled into v_aug [jt, head, 65] with a
    ones column so the PV matmul also emits softmax row-sums for free
  - S^T  [j, i] blocks per head: lhsT = kT j-tile (K=64), rhs = qT i-cols.
    Heads are processed in pairs occupying partitions 0-63 / 64-127, so
    the two matmul streams run concurrently in different PE row-groups.
  - exp via ScalarE activation (scale=1/8) PSUM->SBUF into fp32r P^T;
    causality via block skipping + multiplicative 0/1 mask on diagonal
    blocks (DVE). No max-subtraction needed: logits are O(1).
  - y^T [65, i] = v_aug.T x P^T accumulated over j-tiles in PSUM; row 64
    is the softmax denominator. Normalize: copy sums row, gpsimd
    partition_broadcast, DVE reciprocal, DVE multiply into yT [C, T].
  - out [T, C] = yT-tiles (stationary) x w_proj (moving), DVE copy, DMA.
"""

import numpy as np

import concourse.bass as bass
import concourse.mybir as mybir
import concourse.tile as tile
from concourse import bacc
from concourse.bass_utils import run_bass_kernel_spmd

B, T, C = 8, 1024, 768
NH, HS = 12, 64
NCORES = 8
KT = C // 128            # 6 contraction tiles
NPAIR = NH // 2          # 6 head pairs; head-pair hp covers heads 2hp, 2hp+1
F32 = mybir.dt.float32
F32R = mybir.dt.float32r

_cache = {}


def _build_program(bias_attn: bool, bias_proj: bool):
    nc = bacc.Bacc("TRN2", target_bir_lowering=False, debug=False,
                   num_devices=NCORES)

    xT = nc.dram_tensor("xT", [C, T], F32R, kind="ExternalInput")
    wqk = nc.dram_tensor("wqk", [C, 2 * C], F32R, kind="ExternalInput")
    wv = nc.dram_tensor("wv", [C, C], F32R, kind="ExternalInput")
    wp = nc.dram_tensor("wp", [C, C], F32R, kind="ExternalInput")
    if bias_attn:
        bqk_d = nc.dram_tensor("bqk", [2 * C], F32, kind="ExternalInput")
        bv_d = nc.dram_tensor("bv", [C], F32, kind="ExternalInput")
    if bias_proj:
        bp_d = nc.dram_tensor("bp", [C], F32, kind="ExternalInput")
    out = nc.dram_tensor("out", [T, C], F32, kind="ExternalOutput")

    # 0/1 multiplicative causal mask for diagonal S^T blocks: S^T[j, i]
    # valid where j <= i, i.e. partition index <= free index.
    m01_np = (np.arange(128)[:, None] <= np.arange(128)[None, :]).astype(np.float32)
    m01_d = nc.inline_tensor(m01_np, "m01")

    xT_r = xT.ap().rearrange("(k p) t -> p k t", p=128)
    wqk_r = wqk.ap().rearrange("(k p) m -> p k m", p=128)
    wv_r = wv.ap().rearrange("(k p) m -> p k m", p=128)
    wp_r = wp.ap().rearrange("(k p) m -> p k m", p=128)

    with tile.TileContext(nc) as tc:
        with (
            tc.tile_pool(name="xpool", bufs=1) as xpool,
            tc.tile_pool(name="cpool", bufs=1) as cpool,
            tc.tile_pool(name="wvpool", bufs=1) as wvpool,
            tc.tile_pool(name="vpool", bufs=1) as vpool,
            tc.tile_pool(name="wqkpool", bufs=3) as wqkpool,
            tc.tile_pool(name="qkpool", bufs=2) as qkpool,
            tc.tile_pool(name="ptpool", bufs=3) as ptpool,
            tc.tile_pool(name="ytpool", bufs=1) as ytpool,
            tc.tile_pool(name="wppool", bufs=2) as wppool,
            tc.tile_pool(name="opool", bufs=3) as opool,
            tc.tile_pool(name="smpool", bufs=4) as smpool,
            tc.tile_pool(name="psA", bufs=3, space="PSUM") as psA,
            tc.tile_pool(name="psB", bufs=2, space="PSUM") as psB,
        ):
            # ---- constants / bias staging ----
            m01_s = cpool.tile([128, 128], F32, tag="m01")
            nc.sync.dma_start(m01_s[:], m01_d.ap())
            if bias_attn:
                bqk_s = cpool.tile([128, 12], F32, tag="bqk")
                nc.sync.dma_start(bqk_s[:], bqk_d.ap().rearrange("(m p) -> p m", p=128))
                bv_row = cpool.tile([1, C], F32, tag="bvrow")
                nc.sync.dma_start(bv_row[:], bv_d.ap().rearrange("c -> 1 c"))
                bv_bc = cpool.tile([128, C], F32, tag="bvbc")
                nc.gpsimd.partition_broadcast(bv_bc[:], bv_row[:])
            if bias_proj:
                bp_row = cpool.tile([1, C], F32, tag="bprow")
                nc.sync.dma_start(bp_row[:], bp_d.ap().rearrange("c -> 1 c"))
                bp_bc = cpool.tile([128, C], F32, tag="bpbc")
                nc.gpsimd.partition_broadcast(bp_bc[:], bp_row[:])

            ones3 = cpool.tile([128, NH, 1], F32, tag="ones3")
            nc.vector.memset(ones3[:], 1.0)

            # ---- load x (resident) and w_v ----
            xT_s = xpool.tile([128, KT, T], F32R, tag="xT")
            nc.sync.dma_start(xT_s[:], xT_r)
            wv_s = wvpool.tile([128, KT, C], F32R, tag="wv")
            nc.sync.dma_start(wv_s[:], wv_r)

            # ---- V: token-major, assembled as v_aug[jt, head, 65] ----
            v_aug = vpool.tile([128, 8, NH, HS + 1], F32R, tag="vaug")
            for jt in range(8):
                ps = psA.tile([128, 1024], F32, tag="qkv", bufs=1)
                for off, w in ((0, 512), (512, 256)):
                    for k in range(KT):
                        nc.tensor.matmul(
                            ps[:, off:off + w],
                            xT_s[:, k, jt * 128:(jt + 1) * 128],
                            wv_s[:, k, off:off + w],
                            start=(k == 0), stop=(k == KT - 1),
                        )
                dst = v_aug[:, jt, :, 0:HS]
                src = ps[:, 0:C].rearrange("p (h d) -> p h d", d=HS)
                if bias_attn:
                    nc.vector.tensor_add(
                        dst, src, bv_bc[:].rearrange("p (h d) -> p h d", d=HS))
                else:
                    nc.vector.tensor_copy(dst, src)
                nc.vector.tensor_copy(v_aug[:, jt, :, HS:HS + 1], ones3[:])

            # ---- yT accumulator (written during attention) ----
            yT_s = ytpool.tile([128, KT, T], F32R, tag="yT")

            # ---- per head-pair: QK projection then attention ----
            for hp in range(NPAIR):
                wt = wqkpool.tile([128, KT, 256], F32R, tag="wqk")
                nc.sync.dma_start(wt[:, :, 0:128],
                                  wqk_r[:, :, hp * 128:(hp + 1) * 128])
                nc.sync.dma_start(wt[:, :, 128:256],
                                  wqk_r[:, :, C + hp * 128:C + (hp + 1) * 128])
                qk_t = qkpool.tile([128, 2, T], F32R, tag="qk")
                for part in range(2):  # 0 = q m-tile hp, 1 = k m-tile hp
                    ps = psA.tile([128, 1024], F32, tag="qkv", bufs=1)
                    for nch in range(2):
                        for k in range(KT):
                            nc.tensor.matmul(
                                ps[:, nch * 512:(nch + 1) * 512],
                                wt[:, k, part * 128:part * 128 + 128],
                                xT_s[:, k, nch * 512:(nch + 1) * 512],
                                start=(k == 0), stop=(k == KT - 1),
                            )
                    if bias_attn:
                        nc.vector.tensor_scalar_add(
                            qk_t[:, part, :], ps[:],
                            bqk_s[:, part * 6 + hp:part * 6 + hp + 1])
                    else:
                        nc.vector.tensor_copy(qk_t[:, part, :], ps[:])

                # Both heads of the pair are processed with interleaved
                # instructions: their K=64 S^T matmuls sit in different PE
                # row-groups (partitions 0-63 vs 64-127) and run
                # concurrently when issued back-to-back.
                for c in range(2):  # i-chunk of 512
                    njt = 4 * (c + 1)
                    pts = [[ptpool.tile([128, 4, 512], F32R, tag="pt",
                                        name=f"pt_{hp}_{hl}_{c}_{i}")
                            for i in range(njt // 4)] for hl in range(2)]

                    def pt_ap(hl, jt):
                        return pts[hl][jt // 4][:, jt % 4, :]

                    y_pss = [psB.tile([128, 512], F32, tag="y",
                                      name=f"yps_{hp}_{hl}_{c}")
                             for hl in range(2)]
                    for g in range(njt // 2):
                        sts = [psA.tile([128, 1024], F32, tag="st", bufs=2,
                                        name=f"st_{hp}_{hl}_{c}_{g}")
                               for hl in range(2)]
                        for u in range(2):
                            jt = 2 * g + u
                            lo = max(0, (jt - 4 * c) * 128)
                            for hl in range(2):
                                base = 64 * hl
                                nc.tensor.matmul(
                                    sts[hl][:, u * 512 + lo:(u + 1) * 512],
                                    qk_t[base:base + 64, 1,
                                         jt * 128:(jt + 1) * 128],
                                    qk_t[base:base + 64, 0,
                                         c * 512 + lo:(c + 1) * 512],
                                    start=True, stop=True,
                                )
                        for hl in range(2):
                            nc.scalar.activation(
                                pts[hl][g // 2][:, (g % 2) * 2:(g % 2) * 2 + 2, :],
                                sts[hl][:].rearrange("p (a n) -> p a n", n=512),
                                mybir.ActivationFunctionType.Exp,
                                scale=0.125,
                            )
                    # zero the j>i triangle of diagonal blocks (GpSimd —
                    # keeps the Vector queue clear of the PV critical path)
                    for jt in range(4 * c, njt):
                        dlo = (jt - 4 * c) * 128
                        for hl in range(2):
                            blk = pt_ap(hl, jt)[:, dlo:dlo + 128]
                            nc.gpsimd.tensor_mul(blk, blk, m01_s[:])
                    # PV accumulation (row 64 of y_ps = softmax sums)
                    for jt in range(njt):
                        lo = max(0, (jt - 4 * c) * 128)
                        for hl in range(2):
                            nc.tensor.matmul(
                                y_pss[hl][0:HS + 1, lo:512],
                                v_aug[:, jt, 2 * hp + hl, :],
                                pt_ap(hl, jt)[:, lo:512],
                                start=(jt == 0), stop=(jt == njt - 1),
                                skip_group_check=(jt > 0),
                            )
                    # normalize: yT[h rows, c cols] = y / sums
                    for hl in range(2):
                        base = 64 * hl
                        srow = smpool.tile([1, 512], F32, tag="srow",
                                           name=f"srow_{hp}_{hl}_{c}")
                        nc.vector.tensor_copy(srow[:], y_pss[hl][HS:HS + 1, :])
                        nc.vector.reciprocal_approx_fast(srow[:], srow[:])
                        sbc = smpool.tile([64, 512], F32, tag="sbc",
                                          name=f"sbc_{hp}_{hl}_{c}")
                        nc.gpsimd.partition_broadcast(sbc[:], srow[:])
                        nc.vector.tensor_mul(
                            yT_s[base:base + 64, hp, c * 512:(c + 1) * 512],
                            y_pss[hl][0:HS, :], sbc[:])

            # ---- projection: out = yT.T x w_proj (+ b_proj) ----
            for off, w in ((0, 512), (512, 256)):
                wpt = wppool.tile([128, KT, 512], F32R, tag="wp")
                nc.sync.dma_start(wpt[:, :, 0:w], wp_r[:, :, off:off + w])
                for it in range(8):
                    ps = psA.tile([128, 1024], F32, tag="qkv", bufs=1)
                    for k in range(KT):
                        nc.tensor.matmul(
                            ps[:, 0:w],
                            yT_s[:, k, it * 128:(it + 1) * 128],
                            wpt[:, k, 0:w],
                            start=(k == 0), stop=(k == KT - 1),
                        )
                    ot = opool.tile([128, 512], F32, tag="ot")
                    if bias_proj:
                        nc.vector.tensor_add(ot[:, 0:w], ps[:, 0:w],
                                             bp_bc[:, off:off + w])
                    else:
                        nc.vector.tensor_copy(ot[:, 0:w], ps[:, 0:w])
                    nc.sync.dma_start(out.ap()[it * 128:(it + 1) * 128,
                                               off:off + w], ot[:, 0:w])

    nc.compile()
    return nc


def _get_program(bias_attn, bias_proj):
    key = (bias_attn, bias_proj)
    if key not in _cache:
        _cache[key] = _build_program(bias_attn, bias_proj)
    return _cache[key]


def _prep_inputs(x, w_attn, b_attn, w_proj, b_proj):
    x = np.asarray(x, dtype=np.float32)
    w_attn = np.asarray(w_attn, dtype=np.float32)
    b_attn = np.asarray(b_attn, dtype=np.float32)
    w_proj = np.asarray(w_proj, dtype=np.float32)
    b_proj = np.asarray(b_proj, dtype=np.float32)
    bias_attn = bool(np.any(b_attn))
    bias_proj = bool(np.any(b_proj))
    wqk = np.ascontiguousarray(w_attn[:, :2 * C])
    wv = np.ascontiguousarray(w_attn[:, 2 * C:])
    in_maps = []
    for b in range(NCORES):
        m = {
            "xT": np.ascontiguousarray(x[b].T),
            "wqk": wqk,
            "wv": wv,
            "wp": w_proj,
        }
        if bias_attn:
            m["bqk"] = np.ascontiguousarray(b_attn[:2 * C])
            m["bv"] = np.ascontiguousarray(b_attn[2 * C:])
        if bias_proj:
            m["bp"] = b_proj
        in_maps.append(m)
    return in_maps, bias_attn, bias_proj


def run(x, w_attn, b_attn, w_proj, b_proj, trace=False, tmpdir=None):
    in_maps, bias_attn, bias_proj = _prep_inputs(
        x, w_attn, b_attn, w_proj, b_proj)
    nc = _get_program(bias_attn, bias_proj)
    res = run_bass_kernel_spmd(nc, in_maps, list(range(NCORES)),
                               trace=trace, tmpdir=tmpdir)
    out = np.stack([res.results[i]["out"] for i in range(NCORES)], axis=0)
    return out.astype(np.float32), res


def kernel(x, w_attn, b_attn, w_proj, b_proj):
    out, _ = run(x, w_attn, b_attn, w_proj, b_proj)
    return out


# revision 13
# speedup vs baseline: 1.2234x; 1.2234x over previous
"""Causal self-attention Trainium2 Bass kernel.

Shapes (hardcoded): B=8, T=1024, C=768, NH=12, HS=64.
Sharding: data-parallel over batch — core b computes batch element b.

Per-core dataflow (all matmuls in float32r — tf32-like, ~1.5e-4 rel err):
  - qkT  [2C, T] channel-major  = w_qk.T-tiles (stationary) x xT (moving)
  - v    token-major [T, C], assembled into v_aug [jt, head, 65] with a
    ones column so the PV matmul also emits softmax row-sums for free
  - S^T  [j, i] blocks per head: lhsT = kT j-tile (K=64), rhs = qT i-cols.
    Heads are processed in pairs occupying partitions 0-63 / 64-127, so
    the two matmul streams run concurrently in different PE row-groups.
  - exp via ScalarE activation (scale=1/8) PSUM->SBUF into fp32r P^T;
    causality via block skipping + multiplicative 0/1 mask on diagonal
    blocks (DVE). No max-subtraction needed: logits are O(1).
  - y^T [65, i] = v_aug.T x P^T accumulated over j-tiles in PSUM; row 64
    is the softmax denominator. Normalize: copy sums row, gpsimd
    partition_broadcast, DVE reciprocal, DVE multiply into yT [C, T].
  - out [T, C] = yT-tiles (stationary) x w_proj (moving), DVE copy, DMA.
"""

import numpy as np

import concourse.bass as bass
import concourse.mybir as mybir
import concourse.tile as tile
from concourse import bacc
from concourse.bass_utils import run_bass_kernel_spmd

B, T, C = 8, 1024, 768
NH, HS = 12, 64
NCORES = 8
KT = C // 128            # 6 contraction tiles
NPAIR = NH // 2          # 6 head pairs; head-pair hp covers heads 2hp, 2hp+1
F32 = mybir.dt.float32
F32R = mybir.dt.float32r

_cache = {}


def _build_program(bias_attn: bool, bias_proj: bool):
    nc = bacc.Bacc("TRN2", target_bir_lowering=False, debug=False,
                   num_devices=NCORES)

    xT = nc.dram_tensor("xT", [C, T], F32R, kind="ExternalInput")
    wqk = nc.dram_tensor("wqk", [C, 2 * C], F32R, kind="ExternalInput")
    wv = nc.dram_tensor("wv", [C, C], F32R, kind="ExternalInput")
    wp = nc.dram_tensor("wp", [C, C], F32R, kind="ExternalInput")
    if bias_attn:
        bqk_d = nc.dram_tensor("bqk", [2 * C], F32, kind="ExternalInput")
        bv_d = nc.dram_tensor("bv", [C], F32, kind="ExternalInput")
    if bias_proj:
        bp_d = nc.dram_tensor("bp", [C], F32, kind="ExternalInput")
    out = nc.dram_tensor("out", [T, C], F32, kind="ExternalOutput")

    # 0/1 multiplicative causal mask for diagonal S^T blocks: S^T[j, i]
    # valid where j <= i, i.e. partition index <= free index.
    m01_np = (np.arange(128)[:, None] <= np.arange(128)[None, :]).astype(np.float32)
    m01_d = nc.inline_tensor(m01_np, "m01")

    xT_r = xT.ap().rearrange("(k p) t -> p k t", p=128)
    wqk_r = wqk.ap().rearrange("(k p) m -> p k m", p=128)
    wv_r = wv.ap().rearrange("(k p) m -> p k m", p=128)
    wp_r = wp.ap().rearrange("(k p) m -> p k m", p=128)

    with tile.TileContext(nc) as tc:
        with (
            tc.tile_pool(name="xpool", bufs=1) as xpool,
            tc.tile_pool(name="cpool", bufs=1) as cpool,
            tc.tile_pool(name="wvpool", bufs=1) as wvpool,
            tc.tile_pool(name="vpool", bufs=1) as vpool,
            tc.tile_pool(name="wqkpool", bufs=3) as wqkpool,
            tc.tile_pool(name="qkpool", bufs=2) as qkpool,
            tc.tile_pool(name="ptpool", bufs=3) as ptpool,
            tc.tile_pool(name="ytpool", bufs=1) as ytpool,
            tc.tile_pool(name="wppool", bufs=2) as wppool,
            tc.tile_pool(name="opool", bufs=3) as opool,
            tc.tile_pool(name="smpool", bufs=4) as smpool,
            tc.tile_pool(name="psA", bufs=3, space="PSUM") as psA,
            tc.tile_pool(name="psB", bufs=2, space="PSUM") as psB,
        ):
            # ---- constants / bias staging ----
            m01_s = cpool.tile([128, 128], F32, tag="m01")
            nc.sync.dma_start(m01_s[:], m01_d.ap())
            if bias_attn:
                bqk_s = cpool.tile([128, 12], F32, tag="bqk")
                nc.sync.dma_start(bqk_s[:], bqk_d.ap().rearrange("(m p) -> p m", p=128))
                bv_row = cpool.tile([1, C], F32, tag="bvrow")
                nc.sync.dma_start(bv_row[:], bv_d.ap().rearrange("c -> 1 c"))
                bv_bc = cpool.tile([128, C], F32, tag="bvbc")
                nc.gpsimd.partition_broadcast(bv_bc[:], bv_row[:])
            if bias_proj:
                bp_row = cpool.tile([1, C], F32, tag="bprow")
                nc.sync.dma_start(bp_row[:], bp_d.ap().rearrange("c -> 1 c"))
                bp_bc = cpool.tile([128, C], F32, tag="bpbc")
                nc.gpsimd.partition_broadcast(bp_bc[:], bp_row[:])

            ones3 = cpool.tile([128, NH, 1], F32, tag="ones3")
            nc.vector.memset(ones3[:], 1.0)

            # ---- load x (resident) and w_v ----
            xT_s = xpool.tile([128, KT, T], F32R, tag="xT")
            nc.sync.dma_start(xT_s[:], xT_r)
            wv_s = wvpool.tile([128, KT, C], F32R, tag="wv")
            nc.sync.dma_start(wv_s[:], wv_r)

            # ---- V: token-major, assembled as v_aug[jt, head, 65] ----
            v_aug = vpool.tile([128, 8, NH, HS + 1], F32R, tag="vaug")
            for jt in range(8):
                ps = psA.tile([128, 1024], F32, tag="big")
                for off, w in ((0, 512), (512, 256)):
                    for k in range(KT):
                        nc.tensor.matmul(
                            ps[:, off:off + w],
                            xT_s[:, k, jt * 128:(jt + 1) * 128],
                            wv_s[:, k, off:off + w],
                            start=(k == 0), stop=(k == KT - 1),
                        )
                dst = v_aug[:, jt, :, 0:HS]
                src = ps[:, 0:C].rearrange("p (h d) -> p h d", d=HS)
                if bias_attn:
                    nc.vector.tensor_add(
                        dst, src, bv_bc[:].rearrange("p (h d) -> p h d", d=HS))
                else:
                    nc.vector.tensor_copy(dst, src)
                nc.vector.tensor_copy(v_aug[:, jt, :, HS:HS + 1], ones3[:])

            # ---- yT accumulator (written during attention) ----
            yT_s = ytpool.tile([128, KT, T], F32R, tag="yT")

            # ---- per head-pair: QK projection then attention ----
            for hp in range(NPAIR):
                wt = wqkpool.tile([128, KT, 256], F32R, tag="wqk")
                nc.sync.dma_start(wt[:, :, 0:128],
                                  wqk_r[:, :, hp * 128:(hp + 1) * 128])
                nc.sync.dma_start(wt[:, :, 128:256],
                                  wqk_r[:, :, C + hp * 128:C + (hp + 1) * 128])
                qk_t = qkpool.tile([128, 2, T], F32R, tag="qk")
                for part in range(2):  # 0 = q m-tile hp, 1 = k m-tile hp
                    ps = psA.tile([128, 1024], F32, tag="big")
                    for nch in range(2):
                        for k in range(KT):
                            nc.tensor.matmul(
                                ps[:, nch * 512:(nch + 1) * 512],
                                wt[:, k, part * 128:part * 128 + 128],
                                xT_s[:, k, nch * 512:(nch + 1) * 512],
                                start=(k == 0), stop=(k == KT - 1),
                            )
                    if bias_attn:
                        nc.vector.tensor_scalar_add(
                            qk_t[:, part, :], ps[:],
                            bqk_s[:, part * 6 + hp:part * 6 + hp + 1])
                    else:
                        nc.vector.tensor_copy(qk_t[:, part, :], ps[:])

                # Both heads of the pair are processed with interleaved
                # instructions: their K=64 S^T matmuls sit in different PE
                # row-groups (partitions 0-63 vs 64-127) and run
                # concurrently when issued back-to-back.
                for c in range(2):  # i-chunk of 512
                    njt = 4 * (c + 1)
                    pts = [[ptpool.tile([128, 4, 512], F32R, tag="pt",
                                        name=f"pt_{hp}_{hl}_{c}_{i}")
                            for i in range(njt // 4)] for hl in range(2)]

                    def pt_ap(hl, jt):
                        return pts[hl][jt // 4][:, jt % 4, :]

                    y_pss = [psB.tile([128, 512], F32, tag="y",
                                      name=f"yps_{hp}_{hl}_{c}")
                             for hl in range(2)]

                    def emit_pv(g):
                        # PV for the two j-tiles of group g (both heads)
                        for u in range(2):
                            jt = 2 * g + u
                            lo = max(0, (jt - 4 * c) * 128)
                            for hl in range(2):
                                nc.tensor.matmul(
                                    y_pss[hl][0:HS + 1, lo:512],
                                    v_aug[:, jt, 2 * hp + hl, :],
                                    pt_ap(hl, jt)[:, lo:512],
                                    start=(jt == 0),
                                    stop=(jt == njt - 1),
                                    skip_group_check=(jt > 0),
                                )

                    # Software-pipelined emission: PV of group g-1 is
                    # interleaved between S^T groups so the in-order PE
                    # queue always has work that does not wait on the
                    # ScalarE exp (which drains the S^T PSUM slots).
                    for g in range(njt // 2):
                        sts = [psA.tile([128, 1024], F32, tag="big",
                                        name=f"st_{hp}_{hl}_{c}_{g}")
                               for hl in range(2)]
                        for u in range(2):
                            jt = 2 * g + u
                            lo = max(0, (jt - 4 * c) * 128)
                            for hl in range(2):
                                base = 64 * hl
                                nc.tensor.matmul(
                                    sts[hl][:, u * 512 + lo:(u + 1) * 512],
                                    qk_t[base:base + 64, 1,
                                         jt * 128:(jt + 1) * 128],
                                    qk_t[base:base + 64, 0,
                                         c * 512 + lo:(c + 1) * 512],
                                    start=True, stop=True,
                                )
                        for hl in range(2):
                            nc.scalar.activation(
                                pts[hl][g // 2][:, (g % 2) * 2:(g % 2) * 2 + 2, :],
                                sts[hl][:].rearrange("p (a n) -> p a n", n=512),
                                mybir.ActivationFunctionType.Exp,
                                scale=0.125,
                            )
                        # causal mask of this group's diagonal blocks
                        for u in range(2):
                            jt = 2 * g + u
                            if jt >= 4 * c:
                                dlo = (jt - 4 * c) * 128
                                for hl in range(2):
                                    blk = pt_ap(hl, jt)[:, dlo:dlo + 128]
                                    nc.gpsimd.tensor_mul(blk, blk, m01_s[:])
                        if g >= 1:
                            emit_pv(g - 1)
                    emit_pv(njt // 2 - 1)
                    # normalize: yT[h rows, c cols] = y / sums
                    for hl in range(2):
                        base = 64 * hl
                        srow = smpool.tile([1, 512], F32, tag="srow",
                                           name=f"srow_{hp}_{hl}_{c}")
                        nc.vector.tensor_copy(srow[:], y_pss[hl][HS:HS + 1, :])
                        nc.vector.reciprocal_approx_fast(srow[:], srow[:])
                        sbc = smpool.tile([64, 512], F32, tag="sbc",
                                          name=f"sbc_{hp}_{hl}_{c}")
                        nc.gpsimd.partition_broadcast(sbc[:], srow[:])
                        nc.vector.tensor_mul(
                            yT_s[base:base + 64, hp, c * 512:(c + 1) * 512],
                            y_pss[hl][0:HS, :], sbc[:])

            # ---- projection: out = yT.T x w_proj (+ b_proj) ----
            for off, w in ((0, 512), (512, 256)):
                wpt = wppool.tile([128, KT, 512], F32R, tag="wp")
                nc.sync.dma_start(wpt[:, :, 0:w], wp_r[:, :, off:off + w])
                for it in range(8):
                    ps = psA.tile([128, 1024], F32, tag="big")
                    for k in range(KT):
                        nc.tensor.matmul(
                            ps[:, 0:w],
                            yT_s[:, k, it * 128:(it + 1) * 128],
                            wpt[:, k, 0:w],
                            start=(k == 0), stop=(k == KT - 1),
                        )
                    ot = opool.tile([128, 512], F32, tag="ot")
                    if bias_proj:
                        nc.vector.tensor_add(ot[:, 0:w], ps[:, 0:w],
                                             bp_bc[:, off:off + w])
                    else:
                        nc.vector.tensor_copy(ot[:, 0:w], ps[:, 0:w])
                    nc.sync.dma_start(out.ap()[it * 128:(it + 1) * 128,
                                               off:off + w], ot[:, 0:w])

    nc.compile()
    return nc


def _get_program(bias_attn, bias_proj):
    key = (bias_attn, bias_proj)
    if key not in _cache:
        _cache[key] = _build_program(bias_attn, bias_proj)
    return _cache[key]


def _prep_inputs(x, w_attn, b_attn, w_proj, b_proj):
    x = np.asarray(x, dtype=np.float32)
    w_attn = np.asarray(w_attn, dtype=np.float32)
    b_attn = np.asarray(b_attn, dtype=np.float32)
    w_proj = np.asarray(w_proj, dtype=np.float32)
    b_proj = np.asarray(b_proj, dtype=np.float32)
    bias_attn = bool(np.any(b_attn))
    bias_proj = bool(np.any(b_proj))
    wqk = np.ascontiguousarray(w_attn[:, :2 * C])
    wv = np.ascontiguousarray(w_attn[:, 2 * C:])
    in_maps = []
    for b in range(NCORES):
        m = {
            "xT": np.ascontiguousarray(x[b].T),
            "wqk": wqk,
            "wv": wv,
            "wp": w_proj,
        }
        if bias_attn:
            m["bqk"] = np.ascontiguousarray(b_attn[:2 * C])
            m["bv"] = np.ascontiguousarray(b_attn[2 * C:])
        if bias_proj:
            m["bp"] = b_proj
        in_maps.append(m)
    return in_maps, bias_attn, bias_proj


def run(x, w_attn, b_attn, w_proj, b_proj, trace=False, tmpdir=None):
    in_maps, bias_attn, bias_proj = _prep_inputs(
        x, w_attn, b_attn, w_proj, b_proj)
    nc = _get_program(bias_attn, bias_proj)
    res = run_bass_kernel_spmd(nc, in_maps, list(range(NCORES)),
                               trace=trace, tmpdir=tmpdir)
    out = np.stack([res.results[i]["out"] for i in range(NCORES)], axis=0)
    return out.astype(np.float32), res


def kernel(x, w_attn, b_attn, w_proj, b_proj):
    out, _ = run(x, w_attn, b_attn, w_proj, b_proj)
    return out
